# revision 1
# baseline (speedup 1.0000x reference)
"""Trainium2 Bass kernel for nn_DecoderMinLSTMGNN.

Model (per sample): two MinLSTM layers (D=512) over T=4096 steps, residual,
LayerNorm, projection D->1.  B=8 samples are data-parallel across the 8
NeuronCores (one sample per core).

Per-core layout is channels-major: x^T [D, T] so the time-dim linear
recurrence h_t = a_t*h_{t-1} + (1-a_t)*htilde_t maps onto the VectorE
TensorTensorScan instruction (scan along the free dim, 128 channels per
partition group, carried across 8 time tiles of 512).

Gate math per (group g, time-tile t):
  zf, zi, zh accumulate in PSUM over 4 k-chunks of fp32r matmuls;
  zh additionally gets its bias via a k=1 matmul (bias row x ones row)
  f = sigmoid(zf + bf), i = sigmoid(zi + bi)   (ScalarE, "sigmoid" act set)
  den = f + i                                  (VectorE)
  r = 1/den                                    (ScalarE Reciprocal LUT,
                                                "reciprocal" act set)
  a = f * r                                    (VectorE)
  u' = (a - 1) * zh_psum                       (VectorE scalar_tensor_tensor)
  h  = scan: state = a*state - u'              (VectorE tensor_tensor_scan)
The two ScalarE act-table switches per (t, layer) block cost ~1.3us each.
The VectorE reciprocal instruction is NOT used in the hot loop (it runs at
~6 cycles/elem); the ScalarE Reciprocal LUT runs at streaming rate.

Epilogue: res = h2 + x^T; LN stats + output projection are matmul
reductions against [ones | W_out*ln_g] accumulated into persistent PSUM
banks (partition index = time tile), final LN math is batched on [8,512]
tiles, output DMA'd as [8,512] -> y[4096].
"""

import numpy as np

import concourse.bass as bass
import concourse.mybir as mybir
import concourse.tile as tile
from concourse.bass_utils import run_bass_kernel_spmd

F32 = mybir.dt.float32
F32R = mybir.dt.float32r
AF = mybir.ActivationFunctionType
OP = mybir.AluOpType

B, T, D = 8, 4096, 512
OUT = 1
LN_EPS = 1e-5
TT = 512                 # time-tile size
NT = T // TT             # 8 time tiles
G = D // 128             # 4 channel groups
K = D // 128             # 4 contraction chunks

MAX_WAITS = 1


def _split_excess_waits(nc):
    """walrus in this container rejects >1 semaphore wait per instruction
    ("Too many sync wait commands"); move excess waits onto NoOps."""
    for fn in nc.m.functions:
        for bb in fn.blocks:
            new_list = []
            changed = False
            for inst in bb.instructions:
                si = inst.sync_info
                waits = list(si.on_wait) if si is not None and si.on_wait else []
                if len(waits) > MAX_WAITS:
                    changed = True
                    overflow = waits[:-MAX_WAITS]
                    si.on_wait = waits[-MAX_WAITS:]
                    for j in range(0, len(overflow), MAX_WAITS):
                        new_list.append(mybir.InstNoOp(
                            name=f"{inst.name}-waitsplit-{j}",
                            engine=inst.engine,
                            ins=[], outs=[],
                            sync_info=mybir.SyncInfo(
                                on_wait=overflow[j:j + MAX_WAITS], on_update=[]),
                        ))
                new_list.append(inst)
            if changed:
                bb.instructions[:] = new_list
    return nc


def _act_direct(nc, out, in_, func, bias=0.0, scale=1.0):
    """emit InstActivation directly (bass blocks Reciprocal/Rsqrt)."""
    ins = [nc.scalar.lower_ap(in_)]
    for v in (bias, scale, 0.0):
        if isinstance(v, (int, float)):
            ins.append(mybir.ImmediateValue(dtype=mybir.dt.float32, value=float(v)))
        else:
            ins.append(nc.scalar.lower_ap(v))
    return nc.scalar.add_instruction(
        mybir.InstActivation(
            name=nc.get_next_instruction_name(),
            func=func, ins=ins, outs=[nc.scalar.lower_ap(out)]))


def _build_nc():
    nc = bass.Bass()

    xt_d = nc.dram_tensor("xt", [D, T], F32R, kind="ExternalInput")
    wt_d = nc.dram_tensor("wt", [6, D, D], F32R, kind="ExternalInput")
    # f/i biases per layer: bias[p, layer, {f,i}, g] = b[g*128+p]
    bias_d = nc.dram_tensor("bias", [128, 2, 2, G], F32, kind="ExternalInput")
    # h-gate bias rows (layer, g) -> [1, 128], matmul'd against a ones row
    brow_d = nc.dram_tensor("brow", [2 * G, 128], F32R, kind="ExternalInput")
    ones_d = nc.dram_tensor("ones", [1, TT], F32R, kind="ExternalInput")
    # stats lhsT per (g,t): col t = 1, col 32+t = wg[g*128:(g+1)*128]
    slt_d = nc.dram_tensor("slt", [G, NT, 128, 40], F32R, kind="ExternalInput")
    # S2 lhsT per t: col t = 1
    s2l_d = nc.dram_tensor("s2l", [NT, 128, NT], F32R, kind="ExternalInput")
    epi_d = nc.dram_tensor("epi", [NT, 3], F32, kind="ExternalInput")  # [c0, swg/D, eps]
    out_d = nc.dram_tensor("out", [NT, TT], F32, kind="ExternalOutput")

    with tile.TileContext(nc) as tc:
        with (
            tc.tile_pool(name="const", bufs=1) as const,
            tc.tile_pool(name="xtp", bufs=1) as xtp,
            tc.tile_pool(name="work", bufs=2) as work,
            tc.tile_pool(name="hpool", bufs=2) as hpool,
            tc.tile_pool(name="fin", bufs=1) as fin,
            tc.tile_pool(name="gates_ps", bufs=2, space="PSUM") as gates_ps,
            tc.tile_pool(name="stats_ps", bufs=1, space="PSUM") as stats_ps,
        ):
            # ---- constants ----
            wt_sb = []
            for idx in range(6):
                w = const.tile([128, K, D], F32R, tag=f"wt{idx}")
                nc.sync.dma_start(
                    out=w[:], in_=wt_d[idx].rearrange("(k p) d -> p k d", p=128))
                wt_sb.append(w)
            bias_sb = const.tile([128, 2, 2, G], F32)
            nc.sync.dma_start(out=bias_sb[:], in_=bias_d[:])
            brow_sb = const.tile([1, 2 * G, 128], F32R)
            nc.sync.dma_start(out=brow_sb[:], in_=brow_d[None, :, :])
            ones_sb = const.tile([1, TT], F32R)
            nc.sync.dma_start(out=ones_sb[:], in_=ones_d[:])
            slt_sb = const.tile([128, G, NT, 40], F32R)
            nc.sync.dma_start(
                out=slt_sb[:], in_=slt_d.rearrange("g t p c -> p g t c"))
            s2l_sb = const.tile([128, NT, NT], F32R)
            nc.sync.dma_start(out=s2l_sb[:], in_=s2l_d.rearrange("t p c -> p t c"))
            epi_sb = const.tile([NT, 3], F32)
            nc.sync.dma_start(out=epi_sb[:], in_=epi_d[:])

            # ---- x^T resident tiles, one DMA per (k, t) ----
            xt_sb = [[None] * NT for _ in range(K)]
            for k in range(K):
                for t in range(NT):
                    xx = xtp.tile([128, TT], F32R, tag=f"xt{k}_{t}")
                    nc.sync.dma_start(
                        out=xx[:],
                        in_=xt_d[k * 128:(k + 1) * 128, t * TT:(t + 1) * TT])
                    xt_sb[k][t] = xx

            # persistent stats accumulators (PSUM)
            s13_ps = stats_ps.tile([40, TT], F32, tag="s13")
            s2_ps = stats_ps.tile([NT, TT], F32, tag="s2")
            stats_first = [True]

            h1_sb = [[None] * NT for _ in range(G)]   # layer-1 outputs (F32R)
            h2_sb = [[None] * NT for _ in range(G)]   # layer-2 outputs (F32)

            def layer_tile(layer, t):
                """emit one time-tile of one MinLSTM layer (all 4 groups)"""
                rhs = (xt_sb if layer == 0 else h1_sb)
                h_out = (h1_sb if layer == 0 else h2_sb)
                h_dtype = F32R if layer == 0 else F32
                widx0 = 3 * layer

                pf_l, pi_l, ph_l = [], [], []
                for g in range(G):
                    pf = gates_ps.tile([128, TT], F32, tag="pf")
                    pi = gates_ps.tile([128, TT], F32, tag="pi")
                    ph = gates_ps.tile([128, TT], F32, tag="ph")
                    for gate, ps in ((0, pf), (1, pi), (2, ph)):
                        w = wt_sb[widx0 + gate]
                        for k in range(K):
                            r = rhs[k][t]
                            nc.tensor.matmul(
                                ps[:],
                                w[:, k, g * 128:(g + 1) * 128],
                                r[:] if layer == 0 else r[:].bitcast(F32R),
                                start=(k == 0),
                                stop=(k == K - 1) and (ps is not ph))
                    # h-gate bias via k=1 matmul: ph += bh_row x ones
                    nc.tensor.matmul(
                        ph[:], brow_sb[:, layer * G + g, :], ones_sb[:],
                        start=False, stop=True)
                    pf_l.append(pf)
                    pi_l.append(pi)
                    ph_l.append(ph)

                # phase 1 (sigmoid table): f, i for all groups
                f_l, i_l = [], []
                for g in range(G):
                    f_sb = work.tile([128, TT], F32, tag="f")
                    nc.scalar.activation(
                        f_sb[:], pf_l[g][:], AF.Sigmoid,
                        bias=bias_sb[:, layer, 0, g:g + 1])
                    i_sb = work.tile([128, TT], F32, tag="i")
                    nc.scalar.activation(
                        i_sb[:], pi_l[g][:], AF.Sigmoid,
                        bias=bias_sb[:, layer, 1, g:g + 1])
                    f_l.append(f_sb)
                    i_l.append(i_sb)

                # den = f + i (DVE), then phase 2 (reciprocal table): r = 1/den
                den_l = []
                for g in range(G):
                    den_sb = work.tile([128, TT], F32, tag="den")
                    nc.vector.tensor_add(den_sb[:], f_l[g][:], i_l[g][:])
                    den_l.append(den_sb)
                for g in range(G):
                    r_sb = work.tile([128, TT], F32, tag="r")
                    _act_direct(nc, r_sb[:], den_l[g][:], AF.Reciprocal)
                    a_sb = work.tile([128, TT], F32, tag="a")
                    nc.vector.tensor_mul(a_sb[:], f_l[g][:], r_sb[:])
                    # u' = (a - 1) * zh   (zh read straight from PSUM)
                    up_sb = work.tile([128, TT], F32, tag="up")
                    nc.vector.scalar_tensor_tensor(
                        up_sb[:], a_sb[:], 1.0, ph_l[g][:], OP.subtract, OP.mult)
                    h_sb = hpool.tile([128, TT], h_dtype, tag=f"h{layer}_{g}")
                    init = 0.0 if t == 0 else h_out[g][t - 1][:, TT - 1:TT]
                    nc.vector.tensor_tensor_scan(
                        h_sb[:], a_sb[:], up_sb[:], init, OP.mult, OP.subtract)
                    h_out[g][t] = h_sb

            def epilogue_tile(t):
                """residual + LN/output stats for one time tile"""
                for g in range(G):
                    res = work.tile([128, TT], F32R, tag="res")
                    nc.vector.tensor_add(
                        res[:], h2_sb[g][t][:], xt_sb[g][t][:].bitcast(F32))
                    sq = work.tile([128, TT], F32R, tag="sq")
                    nc.scalar.activation(sq[:], res[:].bitcast(F32), AF.Square)
                    first = stats_first[0]
                    stats_first[0] = False
                    last = (t == NT - 1 and g == G - 1)
                    nc.tensor.matmul(
                        s13_ps[:], slt_sb[:, g, t, :], res[:],
                        start=first, stop=last, skip_group_check=True)
                    nc.tensor.matmul(
                        s2_ps[:], s2l_sb[:, t, :], sq[:],
                        start=first, stop=last, skip_group_check=True)

            # ---- pipeline ----
            for t in range(NT):
                layer_tile(0, t)
                if t >= 1:
                    layer_tile(1, t - 1)
                if t >= 2:
                    epilogue_tile(t - 2)
            layer_tile(1, NT - 1)
            epilogue_tile(NT - 2)
            epilogue_tile(NT - 1)

            # ---- final LN + projection math on [8, 512] ----
            s1 = s13_ps[0:NT, :]
            s3p = s13_ps[32:32 + NT, :]
            s3_sb = fin.tile([NT, TT], F32, tag="s3f")
            nc.scalar.activation(s3_sb[:], s3p, AF.Copy)
            # nn = (s1 * swg/D) - s3
            nn_sb = fin.tile([NT, TT], F32, tag="nn")
            nc.vector.scalar_tensor_tensor(
                nn_sb[:], s1, epi_sb[:, 1:2], s3_sb[:], OP.mult, OP.subtract)
            # s1sq = (s1/D)^2
            s1sq_sb = fin.tile([NT, TT], F32, tag="s1sq")
            nc.scalar.activation(s1sq_sb[:], s1, AF.Square, scale=1.0 / D)
            # v = s2/D - s1sq
            v_sb = fin.tile([NT, TT], F32, tag="v")
            nc.vector.scalar_tensor_tensor(
                v_sb[:], s2_ps[:], 1.0 / D, s1sq_sb[:], OP.mult, OP.subtract)
            # rv = rsqrt(v + eps)  (one more act-table switch, at the very end)
            rv_sb = fin.tile([NT, TT], F32, tag="rv")
            _act_direct(nc, rv_sb[:], v_sb[:], AF.Rsqrt, bias=epi_sb[:, 2:3])
            # pr = (nn * -1) * rv = (s3 - mu*swg) * rv
            pr_sb = fin.tile([NT, TT], F32, tag="pr")
            nc.vector.scalar_tensor_tensor(
                pr_sb[:], nn_sb[:], -1.0, rv_sb[:], OP.mult, OP.mult)
            # out = pr + c0
            o_sb = fin.tile([NT, TT], F32, tag="o")
            nc.scalar.activation(o_sb[:], pr_sb[:], AF.Identity,
                                 bias=epi_sb[:, 0:1])
            nc.sync.dma_start(out=out_d[:], in_=o_sb[:])

    _split_excess_waits(nc)
    return nc


_NC_CACHE = None


def _get_nc():
    global _NC_CACHE
    if _NC_CACHE is None:
        _NC_CACHE = _build_nc()
    return _NC_CACHE


def _host_prep(inputs):
    x = np.asarray(inputs["x"], dtype=np.float32)
    Ws = [inputs[n] for n in ("Wf0", "Wi0", "Wh0", "Wf1", "Wi1", "Wh1")]
    bs = [np.asarray(inputs[n], np.float32) for n in
          ("bf0", "bi0", "bh0", "bf1", "bi1", "bh1")]
    wt_all = np.ascontiguousarray(
        np.stack([np.asarray(w, np.float32).T for w in Ws]))      # [6, din, dout]
    # f/i biases: bias[p, layer, {f,i}, g] = b[g*128+p]
    bias_all = np.zeros((128, 2, 2, G), np.float32)
    for layer in range(2):
        for j in range(2):
            bias_all[:, layer, j, :] = bs[3 * layer + j].reshape(G, 128).T
    # h-gate bias rows: brow[layer*G+g, c] = bh[g*128+c]
    brow = np.zeros((2 * G, 128), np.float32)
    for layer in range(2):
        brow[layer * G:(layer + 1) * G] = bs[3 * layer + 2].reshape(G, 128)
    ones = np.ones((1, TT), np.float32)

    w_out = np.asarray(inputs["W_out"], np.float32).reshape(D)
    ln_g = np.asarray(inputs["ln_g"], np.float32)
    ln_b = np.asarray(inputs["ln_b"], np.float32)
    b_out = np.asarray(inputs["b_out"], np.float32).reshape(())
    wg = w_out * ln_g
    c0 = float(np.dot(w_out, ln_b) + b_out)
    swg = float(wg.sum())

    slt = np.zeros((G, NT, 128, 40), np.float32)
    for g in range(G):
        for t in range(NT):
            slt[g, t, :, t] = 1.0
            slt[g, t, :, 32 + t] = wg[g * 128:(g + 1) * 128]
    s2l = np.zeros((NT, 128, NT), np.float32)
    for t in range(NT):
        s2l[t, :, t] = 1.0
    epi = np.zeros((NT, 3), np.float32)
    epi[:, 0] = c0
    epi[:, 1] = swg / D
    epi[:, 2] = LN_EPS
    return x, wt_all, bias_all, brow, ones, slt, s2l, epi


def _in_maps(inputs):
    x, wt_all, bias_all, brow, ones, slt, s2l, epi = _host_prep(inputs)
    return [
        {
            "xt": np.ascontiguousarray(x[b].T),
            "wt": wt_all, "bias": bias_all, "brow": brow, "ones": ones,
            "slt": slt, "s2l": s2l, "epi": epi,
        }
        for b in range(B)
    ]


def kernel(**inputs):
    nc = _get_nc()
    res = run_bass_kernel_spmd(nc, _in_maps(inputs), list(range(B)))
    out = np.stack([res.results[b]["out"].reshape(T, OUT) for b in range(B)])
    return out.astype(np.float32)


def kernel_traced(**inputs):
    """same as kernel() but returns (output, BassKernelResults) with timing"""
    nc = _get_nc()
    res = run_bass_kernel_spmd(nc, _in_maps(inputs), list(range(B)), trace=True)
    out = np.stack([res.results[b]["out"].reshape(T, OUT) for b in range(B)])
    return out.astype(np.float32), res



# revision 3
# speedup vs baseline: 1.1049x; 1.1049x over previous
"""Trainium2 Bass kernel for nn_DecoderMinLSTMGNN.

Model (per sample): two MinLSTM layers (D=512) over T=4096 steps, residual,
LayerNorm, projection D->1.  B=8 samples data-parallel across 8 NeuronCores.

Channels-major layout: x^T [D, T] in bf16; the time recurrence
h_t = a_t*h_{t-1} + (1-a_t)*htilde_t runs on the VectorE TensorTensorScan
(d0=a in bf16, d1=u' in fp32 — the mixed-dtype scan runs at fp32 speed,
~1.25us per [128,512]; all-bf16 scans hit a 1.8x slow path).

Per (group g, time-tile t):
  PE     : zf, zi, zh accumulate in PSUM over 4 bf16 k-chunk matmuls
           (optionally fp8e4 DoubleRow pairs for layer-0 f/i);
           zh gets its bias via a k=1 ones-row matmul.
  ScalarE: f = sigmoid(zf + bf), i = sigmoid(zi + bi)  (bf16 out)
           r = 1/den  (Reciprocal LUT, bf16)
           epochs batched per t across both layers -> 2 table loads per t
  GpSimd : den = f + i  (plain TensorTensor bf16, SBUF only)
  VectorE: a = f*r (bf16 2x); u' = (a-1)*zh_psum (STT fp32);
           h = scan(a, u') -> bf16

Epilogue: res = h2 + x^T (DVE bf16 2x); sq = res^2 (ScalarE, rides act
epochs); LN/output stats accumulate into one fused PSUM bank (s1|s3 rows
0..39, s2 rows 64..71) via bf16 matmuls; final LN math on [8,512] fp32.
"""

import numpy as np
import ml_dtypes

import concourse.bass as bass
import concourse.mybir as mybir
import concourse.tile as tile
from concourse.bass_utils import run_bass_kernel_spmd

F32 = mybir.dt.float32
BF16 = mybir.dt.bfloat16
FP8 = mybir.dt.float8e4
AF = mybir.ActivationFunctionType
OP = mybir.AluOpType

B, T, D = 8, 4096, 512
OUT = 1
LN_EPS = 1e-5
TT = 512                 # time-tile size
NT = T // TT             # 8 time tiles
G = D // 128             # 4 channel groups
K = D // 128             # 4 contraction chunks

# fp8e4 DoubleRow for layer-0 f/i gate matmuls (rhs = x, host-quantized).
USE_FP8_L0 = False
FP8_SW = 64.0            # weight scale
FP8_SX = 16.0            # activation scale
FP8_DESCALE = 1.0 / (FP8_SW * FP8_SX)

MAX_WAITS = 1


def _split_excess_waits(nc):
    """walrus in this container rejects >1 semaphore wait per instruction;
    move excess waits onto NoOps."""
    for fn in nc.m.functions:
        for bb in fn.blocks:
            new_list = []
            changed = False
            for inst in bb.instructions:
                si = inst.sync_info
                waits = list(si.on_wait) if si is not None and si.on_wait else []
                if len(waits) > MAX_WAITS:
                    changed = True
                    overflow = waits[:-MAX_WAITS]
                    si.on_wait = waits[-MAX_WAITS:]
                    for j in range(0, len(overflow), MAX_WAITS):
                        new_list.append(mybir.InstNoOp(
                            name=f"{inst.name}-waitsplit-{j}",
                            engine=inst.engine,
                            ins=[], outs=[],
                            sync_info=mybir.SyncInfo(
                                on_wait=overflow[j:j + MAX_WAITS], on_update=[]),
                        ))
                new_list.append(inst)
            if changed:
                bb.instructions[:] = new_list
    return nc


def _act_direct(nc, out, in_, func, bias=0.0, scale=1.0):
    """emit InstActivation directly (bass blocks Reciprocal/Rsqrt)."""
    ins = [nc.scalar.lower_ap(in_)]
    for v in (bias, scale, 0.0):
        if isinstance(v, (int, float)):
            ins.append(mybir.ImmediateValue(dtype=mybir.dt.float32, value=float(v)))
        else:
            ins.append(nc.scalar.lower_ap(v))
    return nc.scalar.add_instruction(
        mybir.InstActivation(
            name=nc.get_next_instruction_name(),
            func=func, ins=ins, outs=[nc.scalar.lower_ap(out)]))


def _build_nc():
    nc = bass.Bass()

    xt_d = nc.dram_tensor("xt", [D, T], BF16, kind="ExternalInput")
    wt_d = nc.dram_tensor("wt", [6, D, D], BF16, kind="ExternalInput")
    # f/i biases per layer: bias[p, layer, {f,i}, g] = b[g*128+p]
    bias_d = nc.dram_tensor("bias", [128, 2, 2, G], F32, kind="ExternalInput")
    # h-gate bias rows (layer, g) -> [1, 128], matmul'd against a ones row
    brow_d = nc.dram_tensor("brow", [2 * G, 128], BF16, kind="ExternalInput")
    ones_d = nc.dram_tensor("ones", [1, TT], BF16, kind="ExternalInput")
    # stats lhsT per (g,t): col t = 1, col 32+t = wg[g*128:(g+1)*128]
    slt_d = nc.dram_tensor("slt", [G, NT, 128, 40], BF16, kind="ExternalInput")
    # s2 lhsT per t: col t = 1 (written to stats rows 64..71)
    s2l_d = nc.dram_tensor("s2l", [NT, 128, NT], BF16, kind="ExternalInput")
    epi_d = nc.dram_tensor("epi", [NT, 3], F32, kind="ExternalInput")  # [c0, swg/D, eps]
    out_d = nc.dram_tensor("out", [NT, TT], F32, kind="ExternalOutput")
    if USE_FP8_L0:
        # x for layer-0 f/i: [kpair, p, j, T] fp8, channel d = kpair*256+j*128+p
        x8_d = nc.dram_tensor("x8", [2, 128, 2, T], FP8, kind="ExternalInput")
        w8_d = nc.dram_tensor("w8", [2, 2, 128, 2, 128], FP8, kind="ExternalInput")
        # w8[gate, kpair, p, j, m(g*128..)]  (128-col chunks per g handled by AP)

    with tile.TileContext(nc) as tc:
        with (
            tc.tile_pool(name="const", bufs=1) as const,
            tc.tile_pool(name="xtp", bufs=1) as xtp,
            tc.tile_pool(name="work", bufs=3) as work,
            tc.tile_pool(name="hpool", bufs=2) as hpool,
            tc.tile_pool(name="fin", bufs=1) as fin,
            tc.tile_pool(name="gates_ps", bufs=2, space="PSUM") as gates_ps,
            tc.tile_pool(name="stats_ps", bufs=1, space="PSUM") as stats_ps,
        ):
            # ---- constants ----
            wt_sb = []
            for idx in range(6):
                w = const.tile([128, K, D], BF16, tag=f"wt{idx}")
                nc.sync.dma_start(
                    out=w[:], in_=wt_d[idx].rearrange("(k p) d -> p k d", p=128))
                wt_sb.append(w)
            bias_sb = const.tile([128, 2, 2, G], F32)
            nc.sync.dma_start(out=bias_sb[:], in_=bias_d[:])
            brow_sb = const.tile([1, 2 * G, 128], BF16)
            nc.sync.dma_start(out=brow_sb[:], in_=brow_d[None, :, :])
            ones_sb = const.tile([1, TT], BF16)
            nc.sync.dma_start(out=ones_sb[:], in_=ones_d[:])
            slt_sb = const.tile([128, G, NT, 40], BF16)
            nc.sync.dma_start(
                out=slt_sb[:], in_=slt_d.rearrange("g t p c -> p g t c"))
            s2l_sb = const.tile([128, NT, NT], BF16)
            nc.sync.dma_start(out=s2l_sb[:], in_=s2l_d.rearrange("t p c -> p t c"))
            epi_sb = const.tile([NT, 3], F32)
            nc.sync.dma_start(out=epi_sb[:], in_=epi_d[:])
            if USE_FP8_L0:
                w8_sb = const.tile([128, 2, 2, 2, 128], FP8, tag="w8")
                nc.sync.dma_start(
                    out=w8_sb[:], in_=w8_d.rearrange("gt kp p j m -> p gt kp j m"))

            # ---- x^T resident tiles, one DMA per (k, t) ----
            xt_sb = [[None] * NT for _ in range(K)]
            for k in range(K):
                for t in range(NT):
                    xx = xtp.tile([128, TT], BF16, tag=f"xt{k}_{t}")
                    nc.sync.dma_start(
                        out=xx[:],
                        in_=xt_d[k * 128:(k + 1) * 128, t * TT:(t + 1) * TT])
                    xt_sb[k][t] = xx
            x8_sb = [[None] * NT, [None] * NT]
            if USE_FP8_L0:
                for kp in range(2):
                    for t in range(NT):
                        xx = xtp.tile([128, 2, TT], FP8, tag=f"x8_{kp}_{t}")
                        nc.sync.dma_start(
                            out=xx[:], in_=x8_d[kp, :, :, t * TT:(t + 1) * TT])
                        x8_sb[kp][t] = xx

            # fused stats bank: rows 0..39 = s13, rows 64..71 = s2
            stats_tile = stats_ps.tile([128, TT], F32, tag="stats")
            s13_first = [True]
            s2_first = [True]

            h1_sb = [[None] * NT for _ in range(G)]   # layer-1 outputs (BF16)
            h2_sb = [[None] * NT for _ in range(G)]   # layer-2 outputs (BF16)

            ps_tiles = {}   # (layer, t) -> list of (pf, pi, ph) per g

            def gate_mms(layer, t):
                """emit gate matmuls for one time-tile of one layer"""
                rhs = (xt_sb if layer == 0 else h1_sb)
                widx0 = 3 * layer
                tiles = []
                for g in range(G):
                    pf = gates_ps.tile([128, TT], F32, tag="pf")
                    pi = gates_ps.tile([128, TT], F32, tag="pi")
                    ph = gates_ps.tile([128, TT], F32, tag="ph")
                    if USE_FP8_L0 and layer == 0:
                        for gate, ps in ((0, pf), (1, pi)):
                            for kp in range(2):
                                nc.tensor.matmul(
                                    ps[:],
                                    w8_sb[:, gate, kp, :, g * 128:(g + 1) * 128],
                                    x8_sb[kp][t][:],
                                    start=(kp == 0), stop=(kp == 1),
                                    perf_mode=mybir.MatmulPerfMode.DoubleRow)
                        gates = ((2, ph),)
                    else:
                        gates = ((0, pf), (1, pi), (2, ph))
                    for gate, ps in gates:
                        w = wt_sb[widx0 + gate]
                        for k in range(K):
                            nc.tensor.matmul(
                                ps[:],
                                w[:, k, g * 128:(g + 1) * 128],
                                rhs[k][t][:],
                                start=(k == 0),
                                stop=(k == K - 1) and (ps is not ph))
                    # h-gate bias via k=1 matmul: ph += bh_row x ones
                    nc.tensor.matmul(
                        ph[:], brow_sb[:, layer * G + g, :], ones_sb[:],
                        start=False, stop=True)
                    tiles.append((pf, pi, ph))
                ps_tiles[(layer, t)] = tiles

            def sigmoids(layer, t):
                """sigmoid epoch: f and i (bf16) for all g of one (layer,t)"""
                fis = []
                scale = (FP8_DESCALE if (USE_FP8_L0 and layer == 0) else 1.0)
                for g in range(G):
                    pf, pi, ph = ps_tiles[(layer, t)][g]
                    f_sb = work.tile([128, TT], BF16, tag="f")
                    nc.scalar.activation(
                        f_sb[:], pf[:], AF.Sigmoid,
                        bias=bias_sb[:, layer, 0, g:g + 1], scale=scale)
                    i_sb = work.tile([128, TT], BF16, tag="i")
                    nc.scalar.activation(
                        i_sb[:], pi[:], AF.Sigmoid,
                        bias=bias_sb[:, layer, 1, g:g + 1], scale=scale)
                    fis.append((f_sb, i_sb))
                return fis

            def dens(fis):
                """GpSimd den = f + i (bf16)"""
                out = []
                for f_sb, i_sb in fis:
                    den_sb = work.tile([128, TT], BF16, tag="den")
                    nc.gpsimd.tensor_add(den_sb[:], f_sb[:], i_sb[:])
                    out.append(den_sb)
                return out

            def recips(den_l):
                """reciprocal epoch ops (bf16)"""
                out = []
                for den_sb in den_l:
                    r_sb = work.tile([128, TT], BF16, tag="r")
                    _act_direct(nc, r_sb[:], den_sb[:], AF.Reciprocal)
                    out.append(r_sb)
                return out

            def dve_chain(layer, t, fis, r_l):
                """a = f*r (bf16 2x), up = (a-1)*ph (fp32), h = scan -> bf16"""
                h_out = (h1_sb if layer == 0 else h2_sb)
                for g in range(G):
                    f_sb, _ = fis[g]
                    _, _, ph = ps_tiles[(layer, t)][g]
                    a_sb = work.tile([128, TT], BF16, tag="a")
                    nc.vector.tensor_mul(a_sb[:], f_sb[:], r_l[g][:])
                    up_sb = work.tile([128, TT], F32, tag="up")
                    nc.vector.scalar_tensor_tensor(
                        up_sb[:], a_sb[:], 1.0, ph[:], OP.subtract, OP.mult)
                    h_sb = hpool.tile([128, TT], BF16, tag=f"h{layer}_{g}")
                    init = 0.0 if t == 0 else h_out[g][t - 1][:, TT - 1:TT]
                    nc.vector.tensor_tensor_scan(
                        h_sb[:], a_sb[:], up_sb[:], init, OP.mult, OP.subtract)
                    h_out[g][t] = h_sb

            def epilogue_tile(t):
                """residual + LN/output stats for one time tile"""
                for g in range(G):
                    res = work.tile([128, TT], BF16, tag="res")
                    nc.vector.tensor_add(res[:], h2_sb[g][t][:], xt_sb[g][t][:])
                    sq = work.tile([128, TT], BF16, tag="sq")
                    nc.scalar.activation(sq[:], res[:], AF.Square)
                    last = (t == NT - 1 and g == G - 1)
                    nc.tensor.matmul(
                        stats_tile[0:40, :], slt_sb[:, g, t, :], res[:],
                        start=s13_first[0], stop=last, skip_group_check=True)
                    s13_first[0] = False
                    nc.tensor.matmul(
                        stats_tile[64:72, :], s2l_sb[:, t, :], sq[:],
                        start=s2_first[0], stop=last, skip_group_check=True)
                    s2_first[0] = False

            # ---- pipeline ----
            for t in range(NT + 2):
                if t < NT:
                    gate_mms(0, t)
                if 1 <= t <= NT:
                    gate_mms(1, t - 1)
                # sigmoid epoch (both layers' tiles)
                fis0 = sigmoids(0, t) if t < NT else None
                fis1 = sigmoids(1, t - 1) if 1 <= t <= NT else None
                den0 = dens(fis0) if fis0 else None
                den1 = dens(fis1) if fis1 else None
                # reciprocal epoch
                r0 = recips(den0) if den0 else None
                r1 = recips(den1) if den1 else None
                # DVE chains
                if fis0:
                    dve_chain(0, t, fis0, r0)
                if fis1:
                    dve_chain(1, t - 1, fis1, r1)
                # epilogue (squares ride whatever act epoch is current)
                if 2 <= t:
                    epilogue_tile(t - 2)

            # ---- final LN + projection math on [8, 512] ----
            s1 = stats_tile[0:NT, :]
            s3p = stats_tile[32:32 + NT, :]
            s2 = stats_tile[64:64 + NT, :]
            s3_sb = fin.tile([NT, TT], F32, tag="s3f")
            nc.scalar.activation(s3_sb[:], s3p, AF.Copy)
            # nn = (s1 * swg/D) - s3
            nn_sb = fin.tile([NT, TT], F32, tag="nn")
            nc.vector.scalar_tensor_tensor(
                nn_sb[:], s1, epi_sb[:, 1:2], s3_sb[:], OP.mult, OP.subtract)
            # s1sq = (s1/D)^2
            s1sq_sb = fin.tile([NT, TT], F32, tag="s1sq")
            nc.scalar.activation(s1sq_sb[:], s1, AF.Square, scale=1.0 / D)
            # v = s2/D - s1sq
            v_sb = fin.tile([NT, TT], F32, tag="v")
            nc.vector.scalar_tensor_tensor(
                v_sb[:], s2, 1.0 / D, s1sq_sb[:], OP.mult, OP.subtract)
            # rv = rsqrt(v + eps)
            rv_sb = fin.tile([NT, TT], F32, tag="rv")
            _act_direct(nc, rv_sb[:], v_sb[:], AF.Rsqrt, bias=epi_sb[:, 2:3])
            # pr = (nn * -1) * rv = (s3 - mu*swg) * rv
            pr_sb = fin.tile([NT, TT], F32, tag="pr")
            nc.vector.scalar_tensor_tensor(
                pr_sb[:], nn_sb[:], -1.0, rv_sb[:], OP.mult, OP.mult)
            # out = pr + c0
            o_sb = fin.tile([NT, TT], F32, tag="o")
            nc.scalar.activation(o_sb[:], pr_sb[:], AF.Identity,
                                 bias=epi_sb[:, 0:1])
            nc.sync.dma_start(out=out_d[:], in_=o_sb[:])

    _split_excess_waits(nc)
    return nc


_NC_CACHE = None


def _get_nc():
    global _NC_CACHE
    if _NC_CACHE is None:
        _NC_CACHE = _build_nc()
    return _NC_CACHE


def _host_prep(inputs):
    bf = ml_dtypes.bfloat16
    x = np.asarray(inputs["x"], dtype=np.float32)
    Ws = [np.asarray(inputs[n], np.float32) for n in
          ("Wf0", "Wi0", "Wh0", "Wf1", "Wi1", "Wh1")]
    bs = [np.asarray(inputs[n], np.float32) for n in
          ("bf0", "bi0", "bh0", "bf1", "bi1", "bh1")]
    wt_all = np.ascontiguousarray(
        np.stack([w.T for w in Ws])).astype(bf)                   # [6, din, dout]
    # f/i biases: bias[p, layer, {f,i}, g] = b[g*128+p]
    bias_all = np.zeros((128, 2, 2, G), np.float32)
    for layer in range(2):
        for j in range(2):
            bias_all[:, layer, j, :] = bs[3 * layer + j].reshape(G, 128).T
    # h-gate bias rows: brow[layer*G+g, c] = bh[g*128+c]
    brow = np.zeros((2 * G, 128), np.float32)
    for layer in range(2):
        brow[layer * G:(layer + 1) * G] = bs[3 * layer + 2].reshape(G, 128)
    brow = brow.astype(bf)
    ones = np.ones((1, TT), bf)

    w_out = np.asarray(inputs["W_out"], np.float32).reshape(D)
    ln_g = np.asarray(inputs["ln_g"], np.float32)
    ln_b = np.asarray(inputs["ln_b"], np.float32)
    b_out = np.asarray(inputs["b_out"], np.float32).reshape(())
    wg = w_out * ln_g
    c0 = float(np.dot(w_out, ln_b) + b_out)
    swg = float(np.asarray(wg.astype(bf), np.float32).sum())

    slt = np.zeros((G, NT, 128, 40), np.float32)
    for g in range(G):
        for t in range(NT):
            slt[g, t, :, t] = 1.0
            slt[g, t, :, 32 + t] = wg[g * 128:(g + 1) * 128]
    slt = slt.astype(bf)
    s2l = np.zeros((NT, 128, NT), np.float32)
    for t in range(NT):
        s2l[t, :, t] = 1.0
    s2l = s2l.astype(bf)
    epi = np.zeros((NT, 3), np.float32)
    epi[:, 0] = c0
    epi[:, 1] = swg / D
    epi[:, 2] = LN_EPS

    extra = {}
    if USE_FP8_L0:
        e4 = ml_dtypes.float8_e4m3fn
        # x8[kp, p, j, T]: channel d = kp*256 + j*128 + p, scaled by FP8_SX
        x8 = np.empty((B, 2, 128, 2, T), dtype=e4)
        w8 = np.empty((2, 2, 128, 2, 128), dtype=e4)
        xs = np.clip(x * FP8_SX, -448, 448)
        for kp in range(2):
            for j in range(2):
                lo = kp * 256 + j * 128
                # xs[b, T, d] -> [b, p, T]
                x8[:, kp, :, j, :] = xs[:, :, lo:lo + 128].transpose(0, 2, 1).astype(e4)
        for gate in range(2):
            Wg = np.clip(Ws[gate] * FP8_SW, -448, 448)   # [dout, din]
            for kp in range(2):
                for j in range(2):
                    lo = kp * 256 + j * 128
                    w8[gate, kp, :, j, :] = Wg[:, lo:lo + 128].T.astype(e4)
        extra["w8"] = w8
        extra["x8_per_b"] = x8
    return x, wt_all, bias_all, brow, ones, slt, s2l, epi, extra


def _in_maps(inputs):
    bf = ml_dtypes.bfloat16
    x, wt_all, bias_all, brow, ones, slt, s2l, epi, extra = _host_prep(inputs)
    maps = []
    for b in range(B):
        m = {
            "xt": np.ascontiguousarray(x[b].T).astype(bf),
            "wt": wt_all, "bias": bias_all, "brow": brow, "ones": ones,
            "slt": slt, "s2l": s2l, "epi": epi,
        }
        if USE_FP8_L0:
            m["x8"] = extra["x8_per_b"][b]
            m["w8"] = extra["w8"]
        maps.append(m)
    return maps


def kernel(**inputs):
    nc = _get_nc()
    res = run_bass_kernel_spmd(nc, _in_maps(inputs), list(range(B)))
    out = np.stack([res.results[b]["out"].reshape(T, OUT) for b in range(B)])
    return out.astype(np.float32)


def kernel_traced(**inputs):
    """same as kernel() but returns (output, BassKernelResults) with timing"""
    nc = _get_nc()
    res = run_bass_kernel_spmd(nc, _in_maps(inputs), list(range(B)), trace=True)
    out = np.stack([res.results[b]["out"].reshape(T, OUT) for b in range(B)])
    return out.astype(np.float32), res
